# revision 48
# baseline (speedup 1.0000x reference)
"""Multi-head self-attention (RoPE + causal) Trainium2 Bass kernel, 8-core SPMD.

Problem: B=2, L=2048, D=1024, H=16 heads of Dh=64.
Sharding: each core owns 2 heads x both batches (32 (b,h) pairs / 8 cores = 4).
  - qkv projection: column-parallel (each core computes q/k/v only for its heads)
  - attention: fully local per (b, head)
  - o_proj: row-parallel (each core contracts its 128 ctx dims); host sums the
    8 partial outputs.

Global software pipeline keeps the PE issue queue dense (the tensor engine
only reaches its max clock when issue-to-issue gaps stay near zero):
  - x is DMA'd in large per-(b, l-slice) transfers (few dma_start issues);
    proj for q-block lt starts as soon as slice lt lands.
  - stage (b, lt) interleaves proj/rope/v-transpose of slice lt with the
    attention of q-block lt-1 (cross-wrapping batches), so dependency
    latency in either phase is covered by independent matmuls of the other.
    In the cross-batch stage, units are delayed past the attention steps
    that still read the regions they overwrite.
  - RoPE is fused into the projection: head dims are host-permuted into
    adjacent rotate-pairs (2i, 2i+1) so the partner swap is a DVE
    stream_shuffle; scalar_tensor_tensor ops read the proj PSUM directly
    (bias folded in), with no PE swap matmul and no evac on the chain.
  - scores -> exp -> PV pipelined with lookahead-2; causal masking via
    per-head gpsimd affine_select on the probs; diagonal tiles shrink the
    fully-masked column range out of scores/exp/PV.
  - normalization: ctx is evacuated raw (releasing the PSUM accumulators
    for the next q-block immediately); 1/rowsum is broadcast across
    partitions with a small ones-stationary PE matmul and applied in
    place, keeping gpsimd out of the critical chain.
  - o_proj tasks queue through a cooling list and drain one per PV step as
    PE filler; outputs batch 2 d-chunks per DMA.
  - PSUM: 2 banks proj/rope-bcast/oproj + 4 banks scores + 2 banks ctx.

Matmul dtype: float32r (1 cycle/row at free>=256, ~2.4 GHz when issue-dense).
"""
import sys

import numpy as np

sys.path.insert(0, "/opt/trn_rl_repo")

import concourse.bass as bass  # noqa: E402
import concourse.mybir as mybir  # noqa: E402
import concourse.tile as tile  # noqa: E402
from concourse import bacc  # noqa: E402
from concourse.bass_utils import run_bass_kernel_spmd  # noqa: E402

B, L, D, H, DH = 2, 2048, 1024, 16, 64
NCORES = 8
KC = D // 128          # 8 contraction chunks for the projections
LT = L // 512          # 4 l-slices of 512 (= q-blocks)
NKT = L // 128         # 16 k-tiles of 128
F32 = mybir.dt.float32
F32R = mybir.dt.float32r
IDENT = mybir.ActivationFunctionType.Identity
EXP = mybir.ActivationFunctionType.Exp

_BUILT = None
DEBUG = False
PROFILE_MARKS = False
EMIT_LOG = []


def build():
    nc = bacc.Bacc("TRN2", target_bir_lowering=False, debug=False,
                   num_devices=NCORES)

    def _mark(label):
        if PROFILE_MARKS:
            EMIT_LOG.append((int(nc.get_next_instruction_name()[2:]), label))

    xt_d = nc.dram_tensor("xt", [B, LT, 128, KC, 512], F32R,
                          kind="ExternalInput")
    wq_d = nc.dram_tensor("wq", [128, KC, 128], F32R, kind="ExternalInput")
    wk_d = nc.dram_tensor("wk", [128, KC, 128], F32R, kind="ExternalInput")
    wv_d = nc.dram_tensor("wv", [128, KC, 128], F32R, kind="ExternalInput")
    wo_d = nc.dram_tensor("wo", [128, KC, 128], F32R, kind="ExternalInput")
    cos_d = nc.dram_tensor("cosT", [128, L], F32, kind="ExternalInput")
    sin_d = nc.dram_tensor("sinT", [128, L], F32, kind="ExternalInput")
    # stream_shuffle partner-swap mask: pairs (2i, 2i+1) in each 32-group
    SWAP_MASK = [j + 1 if j % 2 == 0 else j - 1 for j in range(32)]
    ident_d = nc.dram_tensor("ident", [128, 128], F32, kind="ExternalInput")
    bqkv_d = nc.dram_tensor("bqkv", [128, 3], F32, kind="ExternalInput")
    onesr_d = nc.dram_tensor("onesr", [1, 64], F32R, kind="ExternalInput")
    bo_d = nc.dram_tensor("bo", [128, KC], F32, kind="ExternalInput")
    out_d = nc.dram_tensor("out", [B, KC, 128, L], F32, kind="ExternalOutput")
    if DEBUG:
        dq_d = nc.dram_tensor("dq", [128, L], F32R, kind="ExternalOutput")
        dk_d = nc.dram_tensor("dk", [128, L], F32R, kind="ExternalOutput")

    with tile.TileContext(nc) as tc:
        with (
            tc.tile_pool(name="const", bufs=1) as constp,
            tc.tile_pool(name="x", bufs=1) as xp,
            tc.tile_pool(name="qkv", bufs=1) as qkvp,
            tc.tile_pool(name="rope", bufs=1) as ropep,
            tc.tile_pool(name="vsb", bufs=1) as vsbp,
            tc.tile_pool(name="p", bufs=4) as pp,
            tc.tile_pool(name="work", bufs=2) as workp,
            tc.tile_pool(name="c2", bufs=2) as c2p,
            tc.tile_pool(name="outp", bufs=3) as outp,
            tc.tile_pool(name="psP", bufs=2, space="PSUM") as psP,
            tc.tile_pool(name="psS", bufs=2, space="PSUM") as psS,
            tc.tile_pool(name="psC", bufs=1, space="PSUM") as psC,
        ):
            # ---- constants (DMA order = sync-queue order) -----------------
            wq_sb = constp.tile([128, KC, 128], F32R, tag="wq")
            wk_sb = constp.tile([128, KC, 128], F32R, tag="wk")
            wv_sb = constp.tile([128, KC, 128], F32R, tag="wv")
            nc.sync.dma_start(wq_sb[:], wq_d[:])
            bqkv_sb = constp.tile([128, 3], F32, tag="bqkv")

            wo_sb = constp.tile([128, KC, 128], F32R, tag="wo")
            cos_sb = constp.tile([128, L], F32, tag="cos")
            sinpre_sb = constp.tile([128, L], F32, tag="sinpre")
            ident_sb = constp.tile([128, 128], F32, tag="ident")
            bo_sb = constp.tile([128, KC], F32, tag="bo")
            ones_sb = constp.tile([128, NKT], F32, tag="ones")
            nc.gpsimd.memset(ones_sb[:], 1.0)
            onesr_sb = constp.tile([1, 64], F32R, tag="onesr")

            # x slice tiles (one generation, reused across b)
            x_sb = [xp.tile([128, KC, 512], F32R, tag=f"x{lt}",
                            name=f"x{lt}") for lt in range(LT)]

            vT_raw = qkvp.tile([128, L], F32, tag="vraw")
            q_rope = ropep.tile([128, L], F32R, tag="qrope")
            k_rope = ropep.tile([128, L], F32R, tag="krope")
            v_sb = vsbp.tile([128, NKT, 130], F32R, tag="v")
            ctxA = psC.tile([65, 512], F32, tag="ctxA")
            ctxB = psC.tile([65, 512], F32, tag="ctxB")

            pending_o = []
            cooling_o = []
            holdback = [0]
            cur_osb = [None]
            eplg_flip = [0]

            def _emit_oproj(task):
                bb, qb, mt, ctx2 = task
                _mark(f"oproj_b{bb}q{qb}m{mt}")
                ps = psP.tile([128, 512], F32, tag="p")
                nc.tensor.matmul(ps[:], wo_sb[:, mt, :], ctx2[:],
                                 start=True, stop=True)
                if mt % 2 == 0:
                    cur_osb[0] = outp.tile([128, 2, 512], F32, tag="osb",
                                           name="osb")
                osb = cur_osb[0]
                if eplg_flip[0] % 2 == 0:
                    nc.vector.tensor_scalar_add(osb[:, mt % 2, :], ps[:],
                                                bo_sb[:, mt:mt + 1])
                else:
                    nc.scalar.activation(osb[:, mt % 2, :], ps[:], IDENT,
                                         bias=bo_sb[:, mt:mt + 1])
                eplg_flip[0] += 1
                if mt % 2 == 1:
                    qsl = slice(qb * 512, (qb + 1) * 512)
                    nc.sync.dma_start(
                        out_d[bb, mt - 1:mt + 1, :, qsl].rearrange(
                            "a p c -> p a c"),
                        osb[:])

            def _drain_oproj(n):
                for _ in range(min(n, len(pending_o) - holdback[0])):
                    _emit_oproj(pending_o.pop(0))

            # ---- stage emitters -------------------------------------------
            def proj_units(b, lt):
                """Proj/rope/vt units for slice lt of batch b (list of fns)."""
                qsl = slice(lt * 512, (lt + 1) * 512)

                def mk_proj(m, w_sb, raw, bcol):
                    def emit():
                        with nc.named_scope(f"proj_b{b}l{lt}m{m}"):
                            _mark(f"projv_b{b}l{lt}")
                            ps = psP.tile([128, 512], F32, tag="p")
                            for kc in range(KC):
                                nc.tensor.matmul(
                                    ps[:], w_sb[:, kc, :], x_sb[lt][:, kc, :],
                                    start=(kc == 0), stop=(kc == KC - 1))
                            nc.vector.tensor_scalar_add(
                                raw[:, qsl], ps[:],
                                bqkv_sb[:, bcol:bcol + 1])
                    return emit

                def mk_projrope(m, w_sb, rope, bcol):
                    # fused proj + rope: head dims are host-permuted into
                    # adjacent rotate-pairs (2i, 2i+1), so the RoPE partner
                    # swap is a stream_shuffle; bias folds into the
                    # scalar_tensor_tensor ops, which read proj PSUM directly
                    def emit():
                        with nc.named_scope(f"projrope_b{b}l{lt}m{m}"):
                            _mark(f"projrope_b{b}l{lt}m{m}")
                            ps = psP.tile([128, 512], F32, tag="p")
                            for kc in range(KC):
                                nc.tensor.matmul(
                                    ps[:], w_sb[:, kc, :], x_sb[lt][:, kc, :],
                                    start=(kc == 0), stop=(kc == KC - 1))
                            bias = bqkv_sb[:, bcol:bcol + 1]
                            t1 = workp.tile([128, 512], F32, tag="t1")
                            nc.vector.scalar_tensor_tensor(
                                t1[:], ps[:], bias, cos_sb[:, qsl],
                                op0=mybir.AluOpType.add,
                                op1=mybir.AluOpType.mult)
                            u = workp.tile([128, 512], F32, tag="u")
                            nc.vector.scalar_tensor_tensor(
                                u[:], ps[:], bias, sinpre_sb[:, qsl],
                                op0=mybir.AluOpType.add,
                                op1=mybir.AluOpType.mult)
                            t2 = workp.tile([128, 512], F32, tag="t2")
                            nc.vector.stream_shuffle(t2[:], u[:], SWAP_MASK)
                            nc.vector.tensor_add(rope[:, qsl], t1[:], t2[:])
                    return emit

                def mk_vt():
                    def emit():
                        with nc.named_scope(f"vt_b{b}l{lt}"):
                            _mark(f"vt_b{b}l{lt}")
                            ps = psP.tile([128, 512], F32, tag="p")
                            for j in range(4):
                                kt = 4 * lt + j
                                nc.tensor.transpose(
                                    ps[:, j * 128:(j + 1) * 128],
                                    vT_raw[:, kt * 128:(kt + 1) * 128],
                                    ident_sb[:])
                            for j in range(4):
                                kt = 4 * lt + j
                                dst = v_sb[:, kt, :].rearrange(
                                    "p (a c) -> p a c", a=2)
                                src = ps[:, j * 128:(j + 1) * 128].rearrange(
                                    "p (a c) -> p a c", a=2)
                                nc.vector.tensor_copy(dst[:, :, 0:64], src)
                            nc.vector.tensor_copy(
                                v_sb[:, 4 * lt:4 * lt + 4, 64],
                                ones_sb[:, 0:4])
                            nc.vector.tensor_copy(
                                v_sb[:, 4 * lt:4 * lt + 4, 129],
                                ones_sb[:, 0:4])
                    return emit

                return [mk_projrope(0, wq_sb, q_rope, 0),
                        mk_projrope(1, wk_sb, k_rope, 1),
                        mk_proj(2, wv_sb, vT_raw, 2),
                        mk_vt()]

            def attn_steps(b, qb):
                """Attention pipeline steps for q-block qb (list of fns)."""
                nkt = 4 * qb + 4
                qsl = slice(qb * 512, (qb + 1) * 512)
                p2s = {}

                def mk_step(i):
                    def emit():
                        with nc.named_scope(f"attn_b{b}q{qb}i{i}"):
                            _mark(f"attn_b{b}q{qb}i{i}")
                            if i < nkt:
                                kt = i
                                ksl = slice(kt * 128, (kt + 1) * 128)
                                v = kt - 4 * qb
                                # cols < S of this tile are fully masked:
                                # skip them in scores/mask/exp/PV
                                S = 0 if v < 1 else min(128 * v, 256)
                                qsl2 = slice(qb * 512 + S, (qb + 1) * 512)
                                psAB = psS.tile([128, 1024], F32, tag="s")
                                nc.tensor.matmul(psAB[:, S:512],
                                                 k_rope[0:64, ksl],
                                                 q_rope[0:64, qsl2],
                                                 start=True, stop=True)
                                nc.tensor.matmul(psAB[:, 512 + S:1024],
                                                 k_rope[64:128, ksl],
                                                 q_rope[64:128, qsl2],
                                                 start=True, stop=True)
                                p2 = pp.tile([128, 2, 512], F32R, tag="p")
                                ab = psAB[:].rearrange("p (a c) -> p a c",
                                                       a=2)
                                nc.scalar.activation(
                                    p2[:, :, S:], ab[:, :, S:],
                                    EXP, scale=0.125)
                                if v >= 0:
                                    # zero probs where q < k + 128v;
                                    # per-head so PV-A need not wait head B
                                    for hh in (0, 1):
                                        nc.gpsimd.affine_select(
                                            out=p2[:, hh, S:],
                                            in_=p2[:, hh, S:],
                                            compare_op=mybir.AluOpType.is_ge,
                                            fill=0.0, base=S - 128 * v,
                                            pattern=[[1, 512 - S]],
                                            channel_multiplier=-1)
                                p2s[kt] = (p2, S)
                            if i == 3:
                                pending_o.extend(cooling_o)
                                cooling_o.clear()
                            if i >= 2:
                                pv = i - 2
                                p2v, S = p2s.pop(pv)
                                nc.tensor.matmul(
                                    ctxA[:, S:512], v_sb[:, pv, 0:65],
                                    p2v[:, 0, S:],
                                    start=(pv == 0), stop=(pv == nkt - 1))
                                nc.tensor.matmul(
                                    ctxB[:, S:512], v_sb[:, pv, 65:130],
                                    p2v[:, 1, S:],
                                    start=(pv == 0), stop=(pv == nkt - 1))
                                _drain_oproj(1)
                    return emit

                steps = [mk_step(i) for i in range(nkt + 2)]

                def norm():
                    ctx2 = c2p.tile([128, 512], F32R, tag="c2")
                    with nc.named_scope(f"norm_b{b}q{qb}"):
                        _mark(f"norm_b{b}q{qb}")
                        # raw evac first: frees the ctx PSUM banks for the
                        # next q-block's PV without waiting the recip chain
                        ssum2 = workp.tile([1, 1024], F32, tag="ssum")
                        for h, ctx in enumerate((ctxA, ctxB)):
                            nc.vector.tensor_copy(
                                ctx2[h * 64:(h + 1) * 64, :], ctx[0:64, :])
                            nc.vector.tensor_copy(
                                ssum2[:, h * 512:(h + 1) * 512],
                                ctx[64:65, :])
                        rcp2 = workp.tile([1, 1024], F32, tag="rcp")
                        nc.vector.reciprocal_approx_fast(rcp2[:], ssum2[:])
                        rcp2r = workp.tile([1, 1024], F32R, tag="rcpr")
                        nc.vector.tensor_copy(rcp2r[:], rcp2[:])
                        # broadcast 1/sum across partitions on the PE
                        # (ones-stationary matmuls), keeping gpsimd out of
                        # the chain
                        rb1 = psP.tile([128, 512], F32, tag="p")
                        nc.tensor.matmul(rb1[0:64, :], onesr_sb[:],
                                         rcp2r[:, 0:512],
                                         start=True, stop=True)
                        rb2 = psP.tile([128, 512], F32, tag="p")
                        nc.tensor.matmul(rb2[0:64, :], onesr_sb[:],
                                         rcp2r[:, 512:1024],
                                         start=True, stop=True)
                        nc.vector.tensor_mul(ctx2[0:64, :], ctx2[0:64, :],
                                             rb1[0:64, :])
                        rbB = workp.tile([128, 512], F32, tag="rbB")
                        nc.vector.tensor_copy(rbB[64:128, :], rb2[0:64, :])
                        nc.vector.tensor_mul(ctx2[64:128, :],
                                             ctx2[64:128, :],
                                             rbB[64:128, :])
                    cooling_o.extend((b, qb, mt, ctx2) for mt in range(KC))
                steps.append(norm)
                return steps

            # ---- main emission: interleave stage (b, lt) with attention ---
            attn_q = None           # pending attention steps
            for b in range(B):
                for lt in range(LT):
                    if b == 0 and lt == 0:
                        for xp4 in range(4):
                            nc.sync.dma_start(
                                x_sb[0][:, 2 * xp4:2 * xp4 + 2, :],
                                xt_d[0, 0, :, 2 * xp4:2 * xp4 + 2, :])
                        nc.sync.dma_start(wk_sb[:], wk_d[:])
                        nc.sync.dma_start(wv_sb[:], wv_d[:])
                        nc.sync.dma_start(bqkv_sb[:], bqkv_d[:])
                        sl0 = slice(0, 512)
                        nc.sync.dma_start(cos_sb[:, sl0], cos_d[:, sl0])
                        nc.sync.dma_start(sinpre_sb[:, sl0], sin_d[:, sl0])
                        nc.sync.dma_start(x_sb[1][:, 0:4, :],
                                          xt_d[0, 1, :, 0:4, :])
                        nc.sync.dma_start(ident_sb[:], ident_d[:])
                        nc.sync.dma_start(onesr_sb[:], onesr_d[:])
                    elif b == 0 and lt == 1:
                        nc.sync.dma_start(x_sb[1][:, 4:8, :],
                                          xt_d[0, 1, :, 4:8, :])
                        slr = slice(512, L)
                        nc.sync.dma_start(cos_sb[:, slr], cos_d[:, slr])
                        nc.sync.dma_start(sinpre_sb[:, slr], sin_d[:, slr])
                        nc.sync.dma_start(wo_sb[:], wo_d[:])
                        nc.sync.dma_start(bo_sb[:], bo_d[:])
                    else:
                        nc.sync.dma_start(x_sb[lt][:], xt_d[b, lt])

                for lt in range(LT):
                    units = proj_units(b, lt)
                    steps = attn_q if attn_q is not None else []
                    na, nu = len(steps), len(units)
                    ui = 0
                    # cross-batch stage: units(b, lt=0) overwrite
                    # k_rope[0:512] / v_sb[0:4] that attn(b-1, q3) steps 0-5
                    # still read; later-emitted readers would see the new
                    # values, so delay units until those steps have issued
                    u0 = 6 if (lt == 0 and b > 0) else 0
                    if na == 0:
                        for u in units:
                            u()
                            _drain_oproj(1)
                    else:
                        for si, s in enumerate(steps):
                            s()
                            if si >= u0 and ui < nu:
                                units[ui]()
                                ui += 1
                    while ui < nu:
                        units[ui]()
                        ui += 1
                    _drain_oproj(2)
                    attn_q = attn_steps(b, lt)

            if DEBUG:
                nc.sync.dma_start(dq_d[:], q_rope[:])
                nc.sync.dma_start(dk_d[:], k_rope[:])
            # final attention block (b=1, qb=3) + tail drains; hold 3
            # o_proj tasks back as PE filler for the final norm window
            holdback[0] = 3
            for s in attn_q:
                s()
            holdback[0] = 0
            pending_o.extend(cooling_o)
            cooling_o.clear()
            with nc.named_scope("oproj_tail"):
                _drain_oproj(len(pending_o))
    nc.compile()
    return nc


def _host_prep(x, qkv_w, qkv_b, o_w, o_b):
    """Build per-core input maps (all host-side reshapes/transposes)."""
    xt = np.ascontiguousarray(
        x.transpose(0, 2, 1).reshape(B, KC, 128, LT, 512)
        .transpose(0, 3, 2, 1, 4))                      # (B, LT, 128, KC, 512)

    half = DH // 2
    freq = 1.0 / (10000.0 ** (2.0 * np.arange(half, dtype=np.float64) / DH))
    t = np.arange(L, dtype=np.float64)
    freqs = t[:, None] * freq[None, :]                  # (L, 32)
    sinT = np.sin(freqs).T.astype(np.float32)           # (32, L)
    cosT = np.cos(freqs).T.astype(np.float32)

    # head-dim permutation: rotate-pair (i, i+32) -> rows (2i, 2i+1) so the
    # partner swap is a 32-lane stream_shuffle. perm[new] = old.
    perm64 = np.empty(64, dtype=np.int64)
    perm64[0::2] = np.arange(32)
    perm64[1::2] = np.arange(32) + 32
    perm = np.concatenate([perm64, perm64 + 64])        # (128,) per 2 heads

    # cos rows: pair (2i, 2i+1) both use cos_i
    cos64 = np.empty((64, L), dtype=np.float32)
    cos64[0::2] = cosT
    cos64[1::2] = cosT
    cos128 = np.concatenate([cos64, cos64], axis=0)
    # sinpre rows: u[2i] = x1*sin_i (+), u[2i+1] = x2*(-sin_i); after the
    # pair-swap shuffle t2[2i] = u[2i+1] = -x2 sin_i, t2[2i+1] = u[2i]
    sin64 = np.empty((64, L), dtype=np.float32)
    sin64[0::2] = sinT
    sin64[1::2] = -sinT
    sin128 = np.concatenate([sin64, sin64], axis=0)

    ident = np.eye(128, dtype=np.float32)

    in_maps = []
    for c in range(NCORES):
        r = slice(128 * c, 128 * (c + 1))
        wq = np.ascontiguousarray(
            qkv_w[r][perm].T.reshape(KC, 128, 128).transpose(1, 0, 2))
        wk = np.ascontiguousarray(
            qkv_w[D:][r][perm].T.reshape(KC, 128, 128).transpose(1, 0, 2))
        wv = np.ascontiguousarray(
            qkv_w[2 * D:][r].T.reshape(KC, 128, 128).transpose(1, 0, 2))
        wo = np.ascontiguousarray(o_w[:, r].T).reshape(128, KC, 128)
        bqkv = np.stack([qkv_b[r][perm], qkv_b[D:][r][perm],
                         qkv_b[2 * D:][r]],
                        axis=1).astype(np.float32)      # (128, 3)
        # o_b applied by core 0 only (host sums the row-parallel partials)
        if c == 0:
            bo = np.ascontiguousarray(o_b.reshape(KC, 128).T)  # (128, KC)
        else:
            bo = np.zeros((128, KC), dtype=np.float32)
        in_maps.append({
            "xt": xt, "wq": wq, "wk": wk, "wv": wv, "wo": wo,
            "cosT": cos128, "sinT": sin128, "ident": ident,
            "onesr": np.ones((1, 64), dtype=np.float32),
            "bqkv": bqkv, "bo": bo,
        })
    return in_maps


def kernel(x, qkv_w, qkv_b, o_w, o_b, attn_mask, _trace=False):
    global _BUILT
    x = np.asarray(x, dtype=np.float32)
    qkv_w = np.asarray(qkv_w, dtype=np.float32)
    qkv_b = np.asarray(qkv_b, dtype=np.float32)
    o_w = np.asarray(o_w, dtype=np.float32)
    o_b = np.asarray(o_b, dtype=np.float32)
    # attn_mask is all-ones for this problem (spec fill=ones); causal handled
    # on device.

    if _BUILT is None:
        _BUILT = build()
    nc = _BUILT
    in_maps = _host_prep(x, qkv_w, qkv_b, o_w, o_b)
    res = run_bass_kernel_spmd(nc, in_maps, core_ids=list(range(NCORES)),
                               trace=_trace)
    # gather: sum row-parallel partials, then (B, KC, 128, L) -> (B, L, D)
    acc = np.zeros((B, KC, 128, L), dtype=np.float64)
    for r in res.results:
        acc += r["out"].astype(np.float64)
    out = acc.reshape(B, D, L).transpose(0, 2, 1).astype(np.float32)
    if _trace:
        return out, res
    return out
